# revision 14
# baseline (speedup 1.0000x reference)
"""Trainium2 Bass kernel for nn_Mlp_45449343926805 (quantized MLP, 8 cores).

Strategy (v3):
- Data-parallel over batch: x [128,197,384] -> 8 shards of 3152 tokens.
- Everything that is a pure elementwise/layout function of the inputs with a
  host-known scale is done on host (numpy), exactly as the reference does:
  weight quantization, the x activation quant scale, AND the x quantization +
  transpose (qxT shipped as fp16 [D, M] per shard). int8-valued operands are
  exact in fp16.
- Device graph per core:
    GEMM1 (h^T = qw1 @ qxT) with fused bias+GELU out of 4-bank PSUM tiles,
    h stored fp16; |h|max via fused tensor_tensor_reduce; one device
    AllReduce(max) for the h quant scale (the only cross-core dependency);
    requantize h with the fp16 magic-rounding trick (1536.0: fp32->fp16 RNE
    conversion lands exactly on the integer grid, round-half-to-even ==
    jnp.round); GEMM2 with qh^T slices stationary, fc2 bias as a K=1
    ones-row matmul; out = psum * s2 via ScalarE; DMA out.
- A dummy warmup AllReduce at kernel start absorbs collective bring-up and
  launch skew while weights/qxT stream in.
"""

import sys

if "/opt/trn_rl_repo" not in sys.path:
    sys.path.insert(0, "/opt/trn_rl_repo")

import numpy as np

import concourse.bass as bass  # noqa: F401  (registers arch bits)
import concourse.mybir as mybir
import concourse.tile as tile
from concourse import bacc
from concourse import bass_utils

N_CORES = 8
B, S, D, H = 128, 197, 384, 1536
M = (B // N_CORES) * S  # tokens per core = 3152
KD = D // 128  # 3 contraction tiles for fc1
NH = H // 128  # 12 h tiles (also fc2 contraction tiles)
MAGIC = 1536.0  # 1.5*2^10: fp16 round-to-integer magic
R127 = 1.0 / 127.0

FP32 = mybir.dt.float32
FP16 = mybir.dt.float16

# token tiles (partition dim, <=128)
TOKS = [(t, min(128, M - t)) for t in range(0, M, 128)]  # 25
# 512-token chunks (PSUM-bank-width output slices)
CHUNKS = [(c, min(512, M - c)) for c in range(0, M, 512)]  # 7 (last=80)
# GEMM1 blocks: up to 4 chunks -> one [128,2048] psum tile
BLOCKS = [CHUNKS[i : i + 4] for i in range(0, len(CHUNKS), 4)]
# GEMM2 psum groups: up to 4 token tiles -> one [128,2048] psum tile
PSGROUPS = [TOKS[i : i + 4] for i in range(0, len(TOKS), 4)]

AX = mybir.AxisListType.X
OP = mybir.AluOpType
AF = mybir.ActivationFunctionType
GELU_FN = AF.Gelu  # sim_test swaps for Tanh (CoreSim lacks Gelu)
WARMUP_CC = False
SAFE_GROUPS = True  # k-inner contiguous accumulation groups per bank
SAFE_ACT = True  # GELU reads PSUM per 512-col bank slice
USE_TTR = False  # tensor_tensor_reduce for hmax vs plain tensor_reduce


def build_nc(collectives: bool = True):
    nc = bacc.Bacc(
        "TRN2", target_bir_lowering=False, debug=False, num_devices=N_CORES
    )
    qxt_in = nc.dram_tensor("qxt", [D, M], FP16, kind="ExternalInput")
    w1t_in = nc.dram_tensor("w1t", [D, H], FP16, kind="ExternalInput")
    w2t_in = nc.dram_tensor("w2t", [H, D], FP16, kind="ExternalInput")
    b1s1_in = nc.dram_tensor("b1s1", [128, NH], FP32, kind="ExternalInput")
    b2q_in = nc.dram_tensor("b2q", [1, D], FP16, kind="ExternalInput")
    sc_in = nc.dram_tensor("scal", [1, 4], FP32, kind="ExternalInput")
    out = nc.dram_tensor("out", [M, D], FP32, kind="ExternalOutput")

    with tile.TileContext(nc) as tc:
        with (
            tc.tile_pool(name="persist", bufs=1) as pp,
            tc.tile_pool(name="stage", bufs=3) as st,
            tc.tile_pool(name="small", bufs=1) as sm,
            tc.tile_pool(name="ps", bufs=2, space="PSUM") as ps,
            tc.tile_pool(name="dram", bufs=2, space="DRAM") as dram,
        ):
            # ---- persistent weights / constants ----
            # scalar queue: needed earliest (w1t, b1s1, scal)
            w1t_sb = []
            for k in range(KD):
                w1t_k = pp.tile([128, H], FP16, name=f"w1t_{k}", tag=f"w1t_{k}")
                nc.scalar.dma_start(w1t_k[:], w1t_in[128 * k : 128 * (k + 1), :])
                w1t_sb.append(w1t_k)
            b1s1_sb = pp.tile([128, NH], FP32)
            nc.scalar.dma_start(b1s1_sb[:], b1s1_in[:, :])
            sc_row = pp.tile([1, 4], FP32)
            nc.scalar.dma_start(sc_row[:], sc_in[:, :])
            sc_bc = pp.tile([128, 4], FP32)
            nc.gpsimd.partition_broadcast(sc_bc[:], sc_row[:])
            s1_c = sc_bc[:, 1:2]
            sw2_c = sc_bc[:, 2:3]

            # sync queue: qxT in 1024-col pieces (k-interleaved so GEMM1's
            # first block unblocks early), then fc2-side weights
            qxT = [
                pp.tile([128, M], FP16, name=f"qxT_{k}", tag=f"qxT_{k}")
                for k in range(KD)
            ]
            for c0 in range(0, M, 1024):
                cw = min(1024, M - c0)
                for k in range(KD):
                    nc.sync.dma_start(
                        qxT[k][:, c0 : c0 + cw],
                        qxt_in[128 * k : 128 * (k + 1), c0 : c0 + cw],
                    )
            w2t_sb = []
            for k in range(NH):
                w2t_k = pp.tile([128, D], FP16, name=f"w2t_{k}", tag=f"w2t_{k}")
                nc.sync.dma_start(w2t_k[:], w2t_in[128 * k : 128 * (k + 1), :])
                w2t_sb.append(w2t_k)
            b2q_sb = pp.tile([1, D], FP16)
            nc.sync.dma_start(b2q_sb[:], b2q_in[:, :])
            ones1 = pp.tile([1, 128], FP16)
            nc.vector.memset(ones1[:], 1.0)

            h_sb = [
                pp.tile([128, M], FP16, name=f"h_{j}", tag=f"h_{j}")
                for j in range(NH)
            ]
            hmax_cols = pp.tile([128, 24], FP32)

            # ---- warmup collective: absorb CC bring-up + launch skew ----
            if collectives and WARMUP_CC:
                wrm = sm.tile([1, 8], FP32, tag="wrm")
                nc.vector.memset(wrm[:], 0.0)
                warm_in = dram.tile([8], FP32, tag="warm_in")
                warm_out = dram.tile([8], FP32, tag="warm_out")
                nc.sync.dma_start(warm_in[:], wrm[0, :])
                nc.gpsimd.collective_compute(
                    "AllReduce",
                    OP.max,
                    replica_groups=[list(range(N_CORES))],
                    ins=[warm_in[:].opt()],
                    outs=[warm_out[:].opt()],
                )

            # ---- GEMM1: h^T = qw1 @ qxT, fused bias+GELU, h max ----
            col = 0
            for blk in BLOCKS:
                bc0 = blk[0][0]
                bw = sum(cw for _, cw in blk)
                for j in range(NH):
                    psum = ps.tile(
                        [128, 2048], FP32, name=f"ps1_{bc0}_{j}", tag="mm", bufs=2
                    )
                    if SAFE_GROUPS:
                        for ci, (c0, cw) in enumerate(blk):
                            for k in range(KD):
                                nc.tensor.matmul(
                                    psum[:, 512 * ci : 512 * ci + cw],
                                    w1t_sb[k][:, 128 * j : 128 * (j + 1)],
                                    qxT[k][:, c0 : c0 + cw],
                                    start=(k == 0),
                                    stop=(k == KD - 1),
                                )
                    else:
                        for k in range(KD):
                            for ci, (c0, cw) in enumerate(blk):
                                nc.tensor.matmul(
                                    psum[:, 512 * ci : 512 * ci + cw],
                                    w1t_sb[k][:, 128 * j : 128 * (j + 1)],
                                    qxT[k][:, c0 : c0 + cw],
                                    start=(k == 0),
                                    stop=(k == KD - 1),
                                )
                    if SAFE_ACT:
                        for ci, (c0, cw) in enumerate(blk):
                            nc.scalar.activation(
                                h_sb[j][:, c0 : c0 + cw],
                                psum[:, 512 * ci : 512 * ci + cw], GELU_FN,
                                bias=b1s1_sb[:, j : j + 1], scale=s1_c,
                            )
                    else:
                        nc.scalar.activation(
                            h_sb[j][:, bc0 : bc0 + bw], psum[:, :bw], GELU_FN,
                            bias=b1s1_sb[:, j : j + 1], scale=s1_c,
                        )
                    half = bw // 2
                    if USE_TTR:
                        hscr = st.tile(
                            [128, 1024], FP16, name=f"hs_{bc0}_{j}", tag="hs",
                            bufs=2,
                        )
                        # GELU(x) >= -0.17 and the positive max is >> 0.17 for
                        # this model, so max(h) == max(|h|) exactly.
                        nc.vector.tensor_tensor_reduce(
                            hscr[:, :half],
                            h_sb[j][:, bc0 : bc0 + half],
                            h_sb[j][:, bc0 + half : bc0 + bw],
                            1.0,
                            0.0,
                            op0=OP.max,
                            op1=OP.max,
                            accum_out=hmax_cols[:, col : col + 1],
                        )
                    else:
                        nc.vector.tensor_reduce(
                            hmax_cols[:, col : col + 1],
                            h_sb[j][:, bc0 : bc0 + bw],
                            axis=AX, op=OP.max, apply_absolute_value=True,
                        )
                    col += 1

            # ---- AllReduce(max) for the h quant scale ----
            hred = sm.tile([128, 1], FP32, tag="hred")
            nc.vector.tensor_reduce(hred[:], hmax_cols[:, :col], axis=AX, op=OP.max)
            gh = _allreduce_max(nc, dram, sm, hred, collectives, "h")
            s_h = sm.tile([128, 1], FP32, tag="s_h")
            nc.vector.tensor_scalar(s_h[:], gh[:], R127, None, op0=OP.mult)
            i2 = sm.tile([128, 1], FP32, tag="i2")
            nc.vector.reciprocal(i2[:], s_h[:])
            s2 = sm.tile([128, 1], FP32, tag="s2")
            nc.vector.tensor_scalar(s2[:], s_h[:], sw2_c, None, op0=OP.mult)

            # ---- GEMM2: requantize h (fp16 magic), out = (qh@qw2 + b2)*s2 ----
            qh_cur = {}
            cur_chunk = -1
            for pg in PSGROUPS:
                psum = ps.tile(
                    [128, 2048], FP32, name=f"ps2_{pg[0][0]}", tag="mm", bufs=2
                )
                for slot, (t0, tw) in enumerate(pg):
                    g = t0 // 512
                    if g != cur_chunk:
                        c0, cw = CHUNKS[g]
                        qh_cur = {}
                        for k in range(NH):
                            p2 = st.tile(
                                [128, 512], FP16, name=f"p2_{g}_{k}", tag="p2",
                                bufs=4,
                            )
                            nc.vector.tensor_scalar(
                                p2[:, :cw], h_sb[k][:, c0 : c0 + cw], i2[:, 0:1],
                                MAGIC, op0=OP.mult, op1=OP.add,
                            )
                            qh = st.tile(
                                [128, 512], FP16, name=f"qh_{g}_{k}", tag="qh",
                                bufs=24,
                            )
                            nc.vector.tensor_scalar(
                                qh[:, :cw], p2[:, :cw], MAGIC, None,
                                op0=OP.subtract,
                            )
                            qh_cur[k] = qh
                        cur_chunk = g
                    loc = t0 - CHUNKS[g][0]
                    for k in range(NH):
                        nc.tensor.matmul(
                            psum[:tw, 512 * slot : 512 * slot + D],
                            qh_cur[k][:, loc : loc + tw],
                            w2t_sb[k][:, :],
                            start=(k == 0),
                            stop=False,
                        )
                    nc.tensor.matmul(
                        psum[:tw, 512 * slot : 512 * slot + D],
                        ones1[:, :tw],
                        b2q_sb[:, :],
                        start=False,
                        stop=True,
                    )
                for slot, (t0, tw) in enumerate(pg):
                    osb = st.tile(
                        [128, D], FP32, name=f"o_{t0}", tag="o", bufs=4
                    )
                    nc.scalar.activation(
                        osb[:tw, :], psum[:tw, 512 * slot : 512 * slot + D],
                        AF.Copy, bias=0.0, scale=s2[:tw, 0:1],
                    )
                    nc.sync.dma_start(out[t0 : t0 + tw, :], osb[:tw, :])

    nc.compile()
    return nc


def _allreduce_max(nc, dram, sm, vec_col, collectives, name):
    """AllReduce(max) of a [128,1] fp32 column across cores.
    Returns a [128,1] tile holding the global max in every partition."""
    cc_in = dram.tile([128], FP32, name=f"ccin_{name}", tag=f"ccin_{name}")
    cc_out = dram.tile([128], FP32, name=f"ccout_{name}", tag=f"ccout_{name}")
    nc.sync.dma_start(cc_in[:], vec_col[:, 0])
    if collectives:
        nc.gpsimd.collective_compute(
            "AllReduce",
            OP.max,
            replica_groups=[list(range(N_CORES))],
            ins=[cc_in[:].opt()],
            outs=[cc_out[:].opt()],
        )
    else:
        nc.sync.dma_start(cc_out[:], cc_in[:])
    row = sm.tile([1, 128], FP32, name=f"ccrow_{name}", tag=f"ccrow_{name}")
    nc.sync.dma_start(row[:], cc_out[:])
    g1 = sm.tile([1, 1], FP32, name=f"ccg1_{name}", tag=f"ccg1_{name}")
    nc.vector.tensor_reduce(g1[:], row[:], axis=AX, op=OP.max)
    gbc = sm.tile([128, 1], FP32, name=f"ccgb_{name}", tag=f"ccgb_{name}")
    nc.gpsimd.partition_broadcast(gbc[:], g1[:])
    return gbc


# ---------------- host side ----------------

def _quant_weight(w):
    w = np.asarray(w, np.float32)
    s = (np.abs(w).max() / np.float32(127.0)).astype(np.float32)
    q = np.clip(np.round((w / s).astype(np.float32)), -128.0, 127.0)
    return q.astype(np.float32), s


def prep_inputs(x, act_scaling_factor, w1, b1, w2, b2):
    x = np.asarray(x, np.float32).reshape(-1, D)
    s_x = np.float32(np.asarray(act_scaling_factor).reshape(-1)[0])
    qw1, s_w1 = _quant_weight(w1)
    qw2, s_w2 = _quant_weight(w2)
    w1t = np.ascontiguousarray(qw1.T).astype(np.float16)  # [D, H]
    w2t = np.ascontiguousarray(qw2.T).astype(np.float16)  # [H, D]

    # host-side x quantization (pure function of inputs, exactly as the
    # reference: x2 = x*s_x; s_x2 = max|x2|/127; qx = round(clip(x2/s_x2)))
    x2 = (x * s_x).astype(np.float32)
    s_x2 = (np.abs(x2).max() / np.float32(127.0)).astype(np.float32)
    qx = np.round(np.clip((x2 / s_x2).astype(np.float32), -128.0, 127.0))
    qxt = np.ascontiguousarray(qx.reshape(N_CORES, M, D).transpose(0, 2, 1))
    qxt = qxt.astype(np.float16)  # [cores, D, M], int8-valued: exact in fp16
    s1 = (s_w1 * s_x2).astype(np.float32)

    b1s1 = np.ascontiguousarray(
        (np.asarray(b1, np.float32) * s1).reshape(NH, 128).T
    ).astype(np.float32)  # [128, NH]
    b2q = np.asarray(b2, np.float32).reshape(1, D).astype(np.float16)
    scal = np.array([[0.0, s1, s_w2, 0.0]], np.float32)

    in_maps = []
    for c in range(N_CORES):
        in_maps.append(
            {
                "qxt": qxt[c],
                "w1t": w1t,
                "w2t": w2t,
                "b1s1": b1s1,
                "b2q": b2q,
                "scal": scal,
            }
        )
    return in_maps


_NC_CACHE = {}


def get_nc(collectives=True, **_ignored):
    key = collectives
    if key not in _NC_CACHE:
        _NC_CACHE[key] = build_nc(collectives=collectives)
    return _NC_CACHE[key]


def kernel(x, act_scaling_factor, w1, b1, w2, b2):
    in_maps = prep_inputs(x, act_scaling_factor, w1, b1, w2, b2)
    nc = get_nc()
    res = bass_utils.run_bass_kernel_spmd(
        nc, in_maps, core_ids=list(range(N_CORES)), trace=False
    )
    outs = [res.results[c]["out"] for c in range(N_CORES)]
    full = np.concatenate(outs, axis=0).reshape(B, S, D).astype(np.float32)
    return full


if __name__ == "__main__":
    rng = np.random.RandomState(0)
    inputs = {
        "x": rng.randn(B, S, D).astype(np.float32),
        "act_scaling_factor": np.ones(1, np.float32),
        "w1": (rng.randn(H, D) / np.sqrt(D)).astype(np.float32),
        "b1": (0.02 * rng.randn(H)).astype(np.float32),
        "w2": (rng.randn(D, H) / np.sqrt(H)).astype(np.float32),
        "b2": (0.02 * rng.randn(D)).astype(np.float32),
    }
    out = kernel(**inputs)
    print("out", out.shape, out.dtype, float(np.abs(out).max()))


# revision 22
# speedup vs baseline: 1.0222x; 1.0222x over previous
"""Trainium2 Bass kernel for nn_Mlp_45449343926805 (quantized MLP, 8 cores).

Strategy (v3):
- Data-parallel over batch: x [128,197,384] -> 8 shards of 3152 tokens.
- Everything that is a pure elementwise/layout function of the inputs with a
  host-known scale is done on host (numpy), exactly as the reference does:
  weight quantization, the x activation quant scale, AND the x quantization +
  transpose (qxT shipped as fp16 [D, M] per shard). int8-valued operands are
  exact in fp16.
- Device graph per core:
    GEMM1 (h^T = qw1 @ qxT) with fused bias+GELU out of 4-bank PSUM tiles,
    h stored fp16; |h|max via fused tensor_tensor_reduce; one device
    AllReduce(max) for the h quant scale (the only cross-core dependency);
    requantize h with the fp16 magic-rounding trick (1536.0: fp32->fp16 RNE
    conversion lands exactly on the integer grid, round-half-to-even ==
    jnp.round); GEMM2 with qh^T slices stationary, fc2 bias as a K=1
    ones-row matmul; out = psum * s2 via ScalarE; DMA out.
- A dummy warmup AllReduce at kernel start absorbs collective bring-up and
  launch skew while weights/qxT stream in.
"""

import sys

if "/opt/trn_rl_repo" not in sys.path:
    sys.path.insert(0, "/opt/trn_rl_repo")

import numpy as np

import concourse.bass as bass  # noqa: F401  (registers arch bits)
import concourse.mybir as mybir
import concourse.tile as tile
from concourse import bacc
from concourse import bass_utils

N_CORES = 8
B, S, D, H = 128, 197, 384, 1536
M = (B // N_CORES) * S  # tokens per core = 3152
KD = D // 128  # 3 contraction tiles for fc1
NH = H // 128  # 12 h tiles (also fc2 contraction tiles)
MAGIC = 1536.0  # 1.5*2^10: fp16 round-to-integer magic
R127 = 1.0 / 127.0

FP32 = mybir.dt.float32
FP16 = mybir.dt.float16

# token tiles (partition dim, <=128)
TOKS = [(t, min(128, M - t)) for t in range(0, M, 128)]  # 25
# 512-token chunks (PSUM-bank-width output slices)
CHUNKS = [(c, min(512, M - c)) for c in range(0, M, 512)]  # 7 (last=80)
# GEMM1 blocks: up to 4 chunks -> one [128,2048] psum tile
BLOCKS = [CHUNKS[i : i + 4] for i in range(0, len(CHUNKS), 4)]
# GEMM2 psum groups: up to 4 token tiles -> one [128,2048] psum tile
PSGROUPS = [TOKS[i : i + 4] for i in range(0, len(TOKS), 4)]

AX = mybir.AxisListType.X
OP = mybir.AluOpType
AF = mybir.ActivationFunctionType
GELU_FN = AF.Gelu  # sim_test swaps for Tanh (CoreSim lacks Gelu)
WARMUP_CC = True
SAFE_GROUPS = True  # k-inner contiguous accumulation groups per bank
SAFE_ACT = False  # GELU reads PSUM per 512-col bank slice
USE_TTR = False  # tensor_tensor_reduce for hmax vs plain tensor_reduce
LDW_OPT = False  # walrus ldw-opt pass is incompatible with bass ldweights


def _install_ldw_opt():
    if not LDW_OPT:
        return
    if getattr(bass_utils, "_ldw_opt_patched", False):
        return
    orig = bass_utils.run_command

    def run_command(cmd, *a, **kw):
        cmd = [
            "--enable-ldw-opt=true" if c == "--enable-ldw-opt=false" else c
            for c in cmd
        ]
        return orig(cmd, *a, **kw)

    bass_utils.run_command = run_command
    bass_utils._ldw_opt_patched = True


_install_ldw_opt()


def build_nc(collectives: bool = True):
    nc = bacc.Bacc(
        "TRN2", target_bir_lowering=False, debug=False, num_devices=N_CORES
    )
    qxt_in = nc.dram_tensor("qxt", [D, M], FP16, kind="ExternalInput")
    w1t_in = nc.dram_tensor("w1t", [D, H], FP16, kind="ExternalInput")
    w2t_in = nc.dram_tensor("w2t", [H, D], FP16, kind="ExternalInput")
    b1s1_in = nc.dram_tensor("b1s1", [128, NH], FP32, kind="ExternalInput")
    b2q_in = nc.dram_tensor("b2q", [1, D], FP16, kind="ExternalInput")
    sc_in = nc.dram_tensor("scal", [1, 4], FP32, kind="ExternalInput")
    out = nc.dram_tensor("out", [M, D], FP32, kind="ExternalOutput")

    with tile.TileContext(nc) as tc:
        with (
            tc.tile_pool(name="persist", bufs=1) as pp,
            tc.tile_pool(name="stage", bufs=3) as st,
            tc.tile_pool(name="small", bufs=1) as sm,
            tc.tile_pool(name="ps", bufs=2, space="PSUM") as ps,
            tc.tile_pool(name="dram", bufs=2, space="DRAM") as dram,
        ):
            # ---- persistent weights / constants ----
            # scalar queue: needed earliest (w1t, b1s1, scal)
            w1t_sb = []
            for k in range(KD):
                w1t_k = pp.tile([128, H], FP16, name=f"w1t_{k}", tag=f"w1t_{k}")
                nc.scalar.dma_start(w1t_k[:], w1t_in[128 * k : 128 * (k + 1), :])
                w1t_sb.append(w1t_k)
            b1s1_sb = pp.tile([128, NH], FP32)
            nc.scalar.dma_start(b1s1_sb[:], b1s1_in[:, :])
            sc_row = pp.tile([1, 4], FP32)
            nc.scalar.dma_start(sc_row[:], sc_in[:, :])
            sc_bc = pp.tile([128, 4], FP32)
            nc.gpsimd.partition_broadcast(sc_bc[:], sc_row[:])
            s1_c = sc_bc[:, 1:2]
            sw2_c = sc_bc[:, 2:3]

            # sync queue: qxT in 1024-col pieces (k-interleaved so GEMM1's
            # first block unblocks early), then fc2-side weights
            qxT = [
                pp.tile([128, M], FP16, name=f"qxT_{k}", tag=f"qxT_{k}")
                for k in range(KD)
            ]
            for c0 in range(0, M, 1024):
                cw = min(1024, M - c0)
                for k in range(KD):
                    nc.sync.dma_start(
                        qxT[k][:, c0 : c0 + cw],
                        qxt_in[128 * k : 128 * (k + 1), c0 : c0 + cw],
                    )
            w2t_sb = []
            for k in range(NH):
                w2t_k = pp.tile([128, D], FP16, name=f"w2t_{k}", tag=f"w2t_{k}")
                nc.sync.dma_start(w2t_k[:], w2t_in[128 * k : 128 * (k + 1), :])
                w2t_sb.append(w2t_k)
            b2q_sb = pp.tile([1, D], FP16)
            nc.sync.dma_start(b2q_sb[:], b2q_in[:, :])
            ones1 = pp.tile([1, 128], FP16)
            nc.vector.memset(ones1[:], 1.0)

            h_sb = [
                pp.tile([128, M], FP16, name=f"h_{j}", tag=f"h_{j}")
                for j in range(NH)
            ]
            hmax_cols = pp.tile([128, 24], FP32)

            # ---- warmup collective: absorb CC bring-up + launch skew ----
            if collectives and WARMUP_CC:
                wrm = sm.tile([1, 8], FP32, tag="wrm")
                nc.vector.memset(wrm[:], 0.0)
                warm_in = dram.tile([8], FP32, tag="warm_in")
                warm_out = dram.tile([8], FP32, tag="warm_out")
                nc.sync.dma_start(warm_in[:], wrm[0, :])
                nc.gpsimd.collective_compute(
                    "AllReduce",
                    OP.max,
                    replica_groups=[list(range(N_CORES))],
                    ins=[warm_in[:].opt()],
                    outs=[warm_out[:].opt()],
                )

            # ---- GEMM1: h^T = qw1 @ qxT, fused bias+GELU, h max ----
            col = 0
            for blk in BLOCKS:
                bc0 = blk[0][0]
                bw = sum(cw for _, cw in blk)
                for j in range(NH):
                    psum = ps.tile(
                        [128, 2048], FP32, name=f"ps1_{bc0}_{j}", tag="mm", bufs=2
                    )
                    if SAFE_GROUPS:
                        for ci, (c0, cw) in enumerate(blk):
                            for k in range(KD):
                                nc.tensor.matmul(
                                    psum[:, 512 * ci : 512 * ci + cw],
                                    w1t_sb[k][:, 128 * j : 128 * (j + 1)],
                                    qxT[k][:, c0 : c0 + cw],
                                    start=(k == 0),
                                    stop=(k == KD - 1),
                                )
                    else:
                        for k in range(KD):
                            for ci, (c0, cw) in enumerate(blk):
                                nc.tensor.matmul(
                                    psum[:, 512 * ci : 512 * ci + cw],
                                    w1t_sb[k][:, 128 * j : 128 * (j + 1)],
                                    qxT[k][:, c0 : c0 + cw],
                                    start=(k == 0),
                                    stop=(k == KD - 1),
                                )
                    if SAFE_ACT:
                        for ci, (c0, cw) in enumerate(blk):
                            nc.scalar.activation(
                                h_sb[j][:, c0 : c0 + cw],
                                psum[:, 512 * ci : 512 * ci + cw], GELU_FN,
                                bias=b1s1_sb[:, j : j + 1], scale=s1_c,
                            )
                    else:
                        nc.scalar.activation(
                            h_sb[j][:, bc0 : bc0 + bw], psum[:, :bw], GELU_FN,
                            bias=b1s1_sb[:, j : j + 1], scale=s1_c,
                        )
                    half = bw // 2
                    if USE_TTR:
                        hscr = st.tile(
                            [128, 1024], FP16, name=f"hs_{bc0}_{j}", tag="hs",
                            bufs=2,
                        )
                        # GELU(x) >= -0.17 and the positive max is >> 0.17 for
                        # this model, so max(h) == max(|h|) exactly.
                        nc.vector.tensor_tensor_reduce(
                            hscr[:, :half],
                            h_sb[j][:, bc0 : bc0 + half],
                            h_sb[j][:, bc0 + half : bc0 + bw],
                            1.0,
                            0.0,
                            op0=OP.max,
                            op1=OP.max,
                            accum_out=hmax_cols[:, col : col + 1],
                        )
                    else:
                        nc.vector.tensor_reduce(
                            hmax_cols[:, col : col + 1],
                            h_sb[j][:, bc0 : bc0 + bw],
                            axis=AX, op=OP.max, apply_absolute_value=True,
                        )
                    col += 1

            # ---- AllReduce(max) for the h quant scale ----
            hred = sm.tile([128, 1], FP32, tag="hred")
            nc.vector.tensor_reduce(hred[:], hmax_cols[:, :col], axis=AX, op=OP.max)
            gh = _allreduce_max(nc, dram, sm, hred, collectives, "h")
            s_h = sm.tile([128, 1], FP32, tag="s_h")
            nc.vector.tensor_scalar(s_h[:], gh[:], R127, None, op0=OP.mult)
            i2 = sm.tile([128, 1], FP32, tag="i2")
            nc.vector.reciprocal(i2[:], s_h[:])

            s2 = sm.tile([128, 1], FP32, tag="s2")
            nc.vector.tensor_scalar(s2[:], s_h[:], sw2_c, None, op0=OP.mult)

            # ---- GEMM2: requantize h (fp16 magic), out = (qh@qw2 + b2)*s2 ----
            qh_cur = {}
            cur_chunk = -1
            for pg in PSGROUPS:
                psum = ps.tile(
                    [128, 2048], FP32, name=f"ps2_{pg[0][0]}", tag="mm", bufs=2
                )
                for slot, (t0, tw) in enumerate(pg):
                    g = t0 // 512
                    if g != cur_chunk:
                        c0, cw = CHUNKS[g]
                        qh_cur = {}
                        for k in range(NH):
                            p2 = st.tile(
                                [128, 512], FP16, name=f"p2_{g}_{k}", tag="p2",
                                bufs=4,
                            )
                            nc.vector.tensor_scalar(
                                p2[:, :cw], h_sb[k][:, c0 : c0 + cw], i2[:, 0:1],
                                MAGIC, op0=OP.mult, op1=OP.add,
                            )
                            qh = st.tile(
                                [128, 512], FP16, name=f"qh_{g}_{k}", tag="qh",
                                bufs=24,
                            )
                            nc.vector.tensor_scalar(
                                qh[:, :cw], p2[:, :cw], MAGIC, None,
                                op0=OP.subtract,
                            )
                            qh_cur[k] = qh
                        cur_chunk = g
                    loc = t0 - CHUNKS[g][0]
                    for k in range(NH):
                        nc.tensor.matmul(
                            psum[:tw, 512 * slot : 512 * slot + D],
                            qh_cur[k][:, loc : loc + tw],
                            w2t_sb[k][:, :],
                            start=(k == 0),
                            stop=False,
                        )
                    nc.tensor.matmul(
                        psum[:tw, 512 * slot : 512 * slot + D],
                        ones1[:, :tw],
                        b2q_sb[:, :],
                        start=False,
                        stop=True,
                    )
                    osb = st.tile(
                        [128, D], FP32, name=f"o_{t0}", tag="o", bufs=4
                    )
                    nc.scalar.activation(
                        osb[:tw, :], psum[:tw, 512 * slot : 512 * slot + D],
                        AF.Copy, bias=0.0, scale=s2[:tw, 0:1],
                    )
                    nc.sync.dma_start(out[t0 : t0 + tw, :], osb[:tw, :])

    nc.compile()
    return nc


def _allreduce_max(nc, dram, sm, vec_col, collectives, name):
    """AllReduce(max) of a [128,1] fp32 column across cores.
    Returns a [128,1] tile holding the global max in every partition."""
    cc_in = dram.tile([128], FP32, name=f"ccin_{name}", tag=f"ccin_{name}")
    cc_out = dram.tile([128], FP32, name=f"ccout_{name}", tag=f"ccout_{name}")
    nc.sync.dma_start(cc_in[:], vec_col[:, 0])
    if collectives:
        nc.gpsimd.collective_compute(
            "AllReduce",
            OP.max,
            replica_groups=[list(range(N_CORES))],
            ins=[cc_in[:].opt()],
            outs=[cc_out[:].opt()],
        )
    else:
        nc.sync.dma_start(cc_out[:], cc_in[:])
    row = sm.tile([1, 128], FP32, name=f"ccrow_{name}", tag=f"ccrow_{name}")
    nc.sync.dma_start(row[:], cc_out[:])
    g1 = sm.tile([1, 1], FP32, name=f"ccg1_{name}", tag=f"ccg1_{name}")
    nc.vector.tensor_reduce(g1[:], row[:], axis=AX, op=OP.max)
    gbc = sm.tile([128, 1], FP32, name=f"ccgb_{name}", tag=f"ccgb_{name}")
    nc.gpsimd.partition_broadcast(gbc[:], g1[:])
    return gbc


# ---------------- host side ----------------

def _quant_weight(w):
    w = np.asarray(w, np.float32)
    s = (np.abs(w).max() / np.float32(127.0)).astype(np.float32)
    q = np.clip(np.round((w / s).astype(np.float32)), -128.0, 127.0)
    return q.astype(np.float32), s


def prep_inputs(x, act_scaling_factor, w1, b1, w2, b2):
    x = np.asarray(x, np.float32).reshape(-1, D)
    s_x = np.float32(np.asarray(act_scaling_factor).reshape(-1)[0])
    qw1, s_w1 = _quant_weight(w1)
    qw2, s_w2 = _quant_weight(w2)
    w1t = np.ascontiguousarray(qw1.T).astype(np.float16)  # [D, H]
    w2t = np.ascontiguousarray(qw2.T).astype(np.float16)  # [H, D]

    # host-side x quantization (pure function of inputs, exactly as the
    # reference: x2 = x*s_x; s_x2 = max|x2|/127; qx = round(clip(x2/s_x2)))
    x2 = (x * s_x).astype(np.float32)
    s_x2 = (np.abs(x2).max() / np.float32(127.0)).astype(np.float32)
    qx = np.round(np.clip((x2 / s_x2).astype(np.float32), -128.0, 127.0))
    qxt = np.ascontiguousarray(qx.reshape(N_CORES, M, D).transpose(0, 2, 1))
    qxt = qxt.astype(np.float16)  # [cores, D, M], int8-valued: exact in fp16
    s1 = (s_w1 * s_x2).astype(np.float32)

    b1s1 = np.ascontiguousarray(
        (np.asarray(b1, np.float32) * s1).reshape(NH, 128).T
    ).astype(np.float32)  # [128, NH]
    b2q = np.asarray(b2, np.float32).reshape(1, D).astype(np.float16)
    scal = np.array([[0.0, s1, s_w2, 0.0]], np.float32)

    in_maps = []
    for c in range(N_CORES):
        in_maps.append(
            {
                "qxt": qxt[c],
                "w1t": w1t,
                "w2t": w2t,
                "b1s1": b1s1,
                "b2q": b2q,
                "scal": scal,
            }
        )
    return in_maps


_NC_CACHE = {}


def get_nc(collectives=True, **_ignored):
    key = collectives
    if key not in _NC_CACHE:
        _NC_CACHE[key] = build_nc(collectives=collectives)
    return _NC_CACHE[key]


def kernel(x, act_scaling_factor, w1, b1, w2, b2):
    in_maps = prep_inputs(x, act_scaling_factor, w1, b1, w2, b2)
    nc = get_nc()
    res = bass_utils.run_bass_kernel_spmd(
        nc, in_maps, core_ids=list(range(N_CORES)), trace=False
    )
    outs = [res.results[c]["out"] for c in range(N_CORES)]
    full = np.concatenate(outs, axis=0).reshape(B, S, D).astype(np.float32)
    return full


if __name__ == "__main__":
    rng = np.random.RandomState(0)
    inputs = {
        "x": rng.randn(B, S, D).astype(np.float32),
        "act_scaling_factor": np.ones(1, np.float32),
        "w1": (rng.randn(H, D) / np.sqrt(D)).astype(np.float32),
        "b1": (0.02 * rng.randn(H)).astype(np.float32),
        "w2": (rng.randn(D, H) / np.sqrt(H)).astype(np.float32),
        "b2": (0.02 * rng.randn(D)).astype(np.float32),
    }
    out = kernel(**inputs)
    print("out", out.shape, out.dtype, float(np.abs(out).max()))


# revision 26
# speedup vs baseline: 1.0865x; 1.0630x over previous
"""Trainium2 Bass kernel for nn_Mlp_45449343926805 (quantized MLP, 8 cores).

Strategy (v3):
- Data-parallel over batch: x [128,197,384] -> 8 shards of 3152 tokens.
- Everything that is a pure elementwise/layout function of the inputs with a
  host-known scale is done on host (numpy), exactly as the reference does:
  weight quantization, the x activation quant scale, AND the x quantization +
  transpose (qxT shipped as fp16 [D, M] per shard). int8-valued operands are
  exact in fp16.
- Device graph per core:
    GEMM1 (h^T = qw1 @ qxT) with fused bias+GELU out of 4-bank PSUM tiles,
    h stored fp16; |h|max via fused tensor_tensor_reduce; one device
    AllReduce(max) for the h quant scale (the only cross-core dependency);
    requantize h with the fp16 magic-rounding trick (1536.0: fp32->fp16 RNE
    conversion lands exactly on the integer grid, round-half-to-even ==
    jnp.round); GEMM2 with qh^T slices stationary, fc2 bias as a K=1
    ones-row matmul; out = psum * s2 via ScalarE; DMA out.
- A dummy warmup AllReduce at kernel start absorbs collective bring-up and
  launch skew while weights/qxT stream in.
"""

import sys

if "/opt/trn_rl_repo" not in sys.path:
    sys.path.insert(0, "/opt/trn_rl_repo")

import numpy as np

import concourse.bass as bass  # noqa: F401  (registers arch bits)
import concourse.mybir as mybir
import concourse.tile as tile
from concourse import bacc
from concourse import bass_utils

N_CORES = 8
B, S, D, H = 128, 197, 384, 1536
M = (B // N_CORES) * S  # tokens per core = 3152
KD = D // 128  # 3 contraction tiles for fc1
NH = H // 128  # 12 h tiles (also fc2 contraction tiles)
MAGIC = 1536.0  # 1.5*2^10: fp16 round-to-integer magic
R127 = 1.0 / 127.0

FP32 = mybir.dt.float32
FP16 = mybir.dt.float16

# token tiles (partition dim, <=128)
TOKS = [(t, min(128, M - t)) for t in range(0, M, 128)]  # 25
# 512-token chunks (PSUM-bank-width output slices)
CHUNKS = [(c, min(512, M - c)) for c in range(0, M, 512)]  # 7 (last=80)
# GEMM1 blocks: up to 4 chunks -> one [128,2048] psum tile
BLOCKS = [CHUNKS[i : i + 4] for i in range(0, len(CHUNKS), 4)]
# GEMM2 psum groups: up to 4 token tiles -> one [128,2048] psum tile
PSGROUPS = [TOKS[i : i + 4] for i in range(0, len(TOKS), 4)]

AX = mybir.AxisListType.X
OP = mybir.AluOpType
AF = mybir.ActivationFunctionType
GELU_FN = AF.Gelu  # sim_test swaps for Tanh (CoreSim lacks Gelu)
WARMUP_CC = True
SAFE_GROUPS = True  # k-inner contiguous accumulation groups per bank
SAFE_ACT = False  # GELU reads PSUM per 512-col bank slice
USE_TTR = False  # tensor_tensor_reduce for hmax vs plain tensor_reduce
LDW_OPT = False  # walrus ldw-opt pass is incompatible with bass ldweights


def _install_ldw_opt():
    if not LDW_OPT:
        return
    if getattr(bass_utils, "_ldw_opt_patched", False):
        return
    orig = bass_utils.run_command

    def run_command(cmd, *a, **kw):
        cmd = [
            "--enable-ldw-opt=true" if c == "--enable-ldw-opt=false" else c
            for c in cmd
        ]
        return orig(cmd, *a, **kw)

    bass_utils.run_command = run_command
    bass_utils._ldw_opt_patched = True


_install_ldw_opt()


def build_nc(collectives: bool = True):
    nc = bacc.Bacc(
        "TRN2", target_bir_lowering=False, debug=False, num_devices=N_CORES
    )
    qxt_in = nc.dram_tensor("qxt", [D, M], FP16, kind="ExternalInput")
    w1t_in = nc.dram_tensor("w1t", [D, H], FP16, kind="ExternalInput")
    w2t_in = nc.dram_tensor("w2t", [H, D], FP16, kind="ExternalInput")
    b1s1_in = nc.dram_tensor("b1s1", [128, NH], FP32, kind="ExternalInput")
    b2q_in = nc.dram_tensor("b2q", [1, D], FP16, kind="ExternalInput")
    sc_in = nc.dram_tensor("scal", [1, 4], FP32, kind="ExternalInput")
    out = nc.dram_tensor("out", [M, D], FP32, kind="ExternalOutput")

    with tile.TileContext(nc) as tc:
        with (
            tc.tile_pool(name="persist", bufs=1) as pp,
            tc.tile_pool(name="stage", bufs=3) as st,
            tc.tile_pool(name="small", bufs=1) as sm,
            tc.tile_pool(name="ps", bufs=2, space="PSUM") as ps,
            tc.tile_pool(name="dram", bufs=2, space="DRAM") as dram,
        ):
            # ---- persistent weights / constants ----
            # scalar queue: needed earliest (w1t, b1s1, scal)
            w1t_sb = []
            for k in range(KD):
                w1t_k = pp.tile([128, H], FP16, name=f"w1t_{k}", tag=f"w1t_{k}")
                nc.scalar.dma_start(w1t_k[:], w1t_in[128 * k : 128 * (k + 1), :])
                w1t_sb.append(w1t_k)
            b1s1_sb = pp.tile([128, NH], FP32)
            nc.scalar.dma_start(b1s1_sb[:], b1s1_in[:, :])
            sc_row = pp.tile([1, 4], FP32)
            nc.scalar.dma_start(sc_row[:], sc_in[:, :])
            sc_bc = pp.tile([128, 4], FP32)
            nc.gpsimd.partition_broadcast(sc_bc[:], sc_row[:])
            s1_c = sc_bc[:, 1:2]
            sw2_c = sc_bc[:, 2:3]

            # ---- warmup collective: absorb CC bring-up + launch skew ----
            # issued before the bulk DMAs so it runs concurrently with them
            if collectives and WARMUP_CC:
                wrm = sm.tile([1, 8], FP32, tag="wrm")
                nc.vector.memset(wrm[:], 0.0)
                warm_in = dram.tile([8], FP32, tag="warm_in")
                warm_out = dram.tile([8], FP32, tag="warm_out")
                nc.sync.dma_start(warm_in[:], wrm[0, :])
                nc.gpsimd.collective_compute(
                    "AllReduce",
                    OP.max,
                    replica_groups=[list(range(N_CORES))],
                    ins=[warm_in[:].opt()],
                    outs=[warm_out[:].opt()],
                )

            # sync queue: qxT in 1024-col pieces (k-interleaved so GEMM1's
            # first block unblocks early), then fc2-side weights
            qxT = [
                pp.tile([128, M], FP16, name=f"qxT_{k}", tag=f"qxT_{k}")
                for k in range(KD)
            ]
            for c0 in range(0, M, 1024):
                cw = min(1024, M - c0)
                for k in range(KD):
                    nc.sync.dma_start(
                        qxT[k][:, c0 : c0 + cw],
                        qxt_in[128 * k : 128 * (k + 1), c0 : c0 + cw],
                    )
            w2t_sb = []
            for k in range(NH):
                w2t_k = pp.tile([128, D], FP16, name=f"w2t_{k}", tag=f"w2t_{k}")
                nc.sync.dma_start(w2t_k[:], w2t_in[128 * k : 128 * (k + 1), :])
                w2t_sb.append(w2t_k)
            b2q_sb = pp.tile([1, D], FP16)
            nc.sync.dma_start(b2q_sb[:], b2q_in[:, :])
            ones1 = pp.tile([1, 128], FP16)
            nc.vector.memset(ones1[:], 1.0)

            h_sb = [
                pp.tile([128, M], FP16, name=f"h_{j}", tag=f"h_{j}")
                for j in range(NH)
            ]
            hmax_cols = pp.tile([128, 24], FP32)

            # ---- GEMM1: h^T = qw1 @ qxT, fused bias+GELU, h max ----
            col = 0
            for blk in BLOCKS:
                bc0 = blk[0][0]
                bw = sum(cw for _, cw in blk)
                for j in range(NH):
                    psum = ps.tile(
                        [128, 2048], FP32, name=f"ps1_{bc0}_{j}", tag="mm", bufs=2
                    )
                    if SAFE_GROUPS:
                        for ci, (c0, cw) in enumerate(blk):
                            for k in range(KD):
                                nc.tensor.matmul(
                                    psum[:, 512 * ci : 512 * ci + cw],
                                    w1t_sb[k][:, 128 * j : 128 * (j + 1)],
                                    qxT[k][:, c0 : c0 + cw],
                                    start=(k == 0),
                                    stop=(k == KD - 1),
                                )
                    else:
                        for k in range(KD):
                            for ci, (c0, cw) in enumerate(blk):
                                nc.tensor.matmul(
                                    psum[:, 512 * ci : 512 * ci + cw],
                                    w1t_sb[k][:, 128 * j : 128 * (j + 1)],
                                    qxT[k][:, c0 : c0 + cw],
                                    start=(k == 0),
                                    stop=(k == KD - 1),
                                )
                    if SAFE_ACT:
                        for ci, (c0, cw) in enumerate(blk):
                            nc.scalar.activation(
                                h_sb[j][:, c0 : c0 + cw],
                                psum[:, 512 * ci : 512 * ci + cw], GELU_FN,
                                bias=b1s1_sb[:, j : j + 1], scale=s1_c,
                            )
                    else:
                        nc.scalar.activation(
                            h_sb[j][:, bc0 : bc0 + bw], psum[:, :bw], GELU_FN,
                            bias=b1s1_sb[:, j : j + 1], scale=s1_c,
                        )
                    half = bw // 2
                    if USE_TTR:
                        hscr = st.tile(
                            [128, 1024], FP16, name=f"hs_{bc0}_{j}", tag="hs",
                            bufs=2,
                        )
                        # GELU(x) >= -0.17 and the positive max is >> 0.17 for
                        # this model, so max(h) == max(|h|) exactly.
                        nc.vector.tensor_tensor_reduce(
                            hscr[:, :half],
                            h_sb[j][:, bc0 : bc0 + half],
                            h_sb[j][:, bc0 + half : bc0 + bw],
                            1.0,
                            0.0,
                            op0=OP.max,
                            op1=OP.max,
                            accum_out=hmax_cols[:, col : col + 1],
                        )
                    else:
                        nc.vector.tensor_reduce(
                            hmax_cols[:, col : col + 1],
                            h_sb[j][:, bc0 : bc0 + bw],
                            axis=AX, op=OP.max, apply_absolute_value=True,
                        )
                    col += 1

            # ---- AllReduce(max) for the h quant scale ----
            hred = sm.tile([128, 1], FP32, tag="hred")
            nc.vector.tensor_reduce(hred[:], hmax_cols[:, :col], axis=AX, op=OP.max)
            gh = _allreduce_max(nc, dram, sm, hred, collectives, "h")
            s_h = sm.tile([128, 1], FP32, tag="s_h")
            nc.vector.tensor_scalar(s_h[:], gh[:], R127, None, op0=OP.mult)
            i2 = sm.tile([128, 1], FP32, tag="i2")
            nc.vector.reciprocal(i2[:], s_h[:])

            s2 = sm.tile([128, 1], FP32, tag="s2")
            nc.vector.tensor_scalar(s2[:], s_h[:], sw2_c, None, op0=OP.mult)

            # ---- GEMM2: requantize h (fp16 magic), out = (qh@qw2 + b2)*s2 ----
            qh_cur = {}
            cur_chunk = -1
            for pg in PSGROUPS:
                psum = ps.tile(
                    [128, 2048], FP32, name=f"ps2_{pg[0][0]}", tag="mm", bufs=2
                )
                for slot, (t0, tw) in enumerate(pg):
                    g = t0 // 512
                    if g != cur_chunk:
                        c0, cw = CHUNKS[g]
                        qh_cur = {}
                        for k in range(NH):
                            p2 = st.tile(
                                [128, 512], FP16, name=f"p2_{g}_{k}", tag="p2",
                                bufs=4,
                            )
                            if k < 4:
                                # offload 1/3 of requant pass1 to ScalarE:
                                # Copy(i2*h + MAGIC) with fp16 RNE output
                                nc.scalar.activation(
                                    p2[:, :cw], h_sb[k][:, c0 : c0 + cw],
                                    AF.Copy, bias=MAGIC, scale=i2[:, 0:1],
                                )
                            else:
                                nc.vector.tensor_scalar(
                                    p2[:, :cw], h_sb[k][:, c0 : c0 + cw],
                                    i2[:, 0:1], MAGIC, op0=OP.mult, op1=OP.add,
                                )
                            qh = st.tile(
                                [128, 512], FP16, name=f"qh_{g}_{k}", tag="qh",
                                bufs=24,
                            )
                            nc.vector.tensor_scalar(
                                qh[:, :cw], p2[:, :cw], MAGIC, None,
                                op0=OP.subtract,
                            )
                            qh_cur[k] = qh
                        cur_chunk = g
                    loc = t0 - CHUNKS[g][0]
                    for k in range(NH):
                        nc.tensor.matmul(
                            psum[:tw, 512 * slot : 512 * slot + D],
                            qh_cur[k][:, loc : loc + tw],
                            w2t_sb[k][:, :],
                            start=(k == 0),
                            stop=False,
                        )
                    nc.tensor.matmul(
                        psum[:tw, 512 * slot : 512 * slot + D],
                        ones1[:, :tw],
                        b2q_sb[:, :],
                        start=False,
                        stop=True,
                    )
                    osb = st.tile(
                        [128, D], FP32, name=f"o_{t0}", tag="o", bufs=4
                    )
                    nc.scalar.activation(
                        osb[:tw, :], psum[:tw, 512 * slot : 512 * slot + D],
                        AF.Copy, bias=0.0, scale=s2[:tw, 0:1],
                    )
                    nc.sync.dma_start(out[t0 : t0 + tw, :], osb[:tw, :])

    nc.compile()
    return nc


def _allreduce_max(nc, dram, sm, vec_col, collectives, name):
    """AllReduce(max) of a [128,1] fp32 column across cores.
    Returns a [128,1] tile holding the global max in every partition."""
    cc_in = dram.tile([128], FP32, name=f"ccin_{name}", tag=f"ccin_{name}")
    nc.sync.dma_start(cc_in[:], vec_col[:, 0])
    if collectives:
        # AllGather + local max: skips the reduce-scatter phase of AllReduce
        cc_out = dram.tile(
            [128 * N_CORES], FP32, name=f"ccout_{name}", tag=f"ccout_{name}"
        )
        nc.gpsimd.collective_compute(
            "AllGather",
            OP.bypass,
            replica_groups=[list(range(N_CORES))],
            ins=[cc_in[:].opt()],
            outs=[cc_out[:].opt()],
        )
        row = sm.tile(
            [1, 128 * N_CORES], FP32, name=f"ccrow_{name}", tag=f"ccrow_{name}"
        )
        nc.sync.dma_start(row[:], cc_out[:])
    else:
        cc_out = dram.tile([128], FP32, name=f"ccout_{name}", tag=f"ccout_{name}")
        nc.sync.dma_start(cc_out[:], cc_in[:])
        row = sm.tile([1, 128], FP32, name=f"ccrow_{name}", tag=f"ccrow_{name}")
        nc.sync.dma_start(row[:], cc_out[:])
    g1 = sm.tile([1, 1], FP32, name=f"ccg1_{name}", tag=f"ccg1_{name}")
    nc.vector.tensor_reduce(g1[:], row[:], axis=AX, op=OP.max)
    gbc = sm.tile([128, 1], FP32, name=f"ccgb_{name}", tag=f"ccgb_{name}")
    nc.gpsimd.partition_broadcast(gbc[:], g1[:])
    return gbc


# ---------------- host side ----------------

def _quant_weight(w):
    w = np.asarray(w, np.float32)
    s = (np.abs(w).max() / np.float32(127.0)).astype(np.float32)
    q = np.clip(np.round((w / s).astype(np.float32)), -128.0, 127.0)
    return q.astype(np.float32), s


def prep_inputs(x, act_scaling_factor, w1, b1, w2, b2):
    x = np.asarray(x, np.float32).reshape(-1, D)
    s_x = np.float32(np.asarray(act_scaling_factor).reshape(-1)[0])
    qw1, s_w1 = _quant_weight(w1)
    qw2, s_w2 = _quant_weight(w2)
    w1t = np.ascontiguousarray(qw1.T).astype(np.float16)  # [D, H]
    w2t = np.ascontiguousarray(qw2.T).astype(np.float16)  # [H, D]

    # host-side x quantization (pure function of inputs, exactly as the
    # reference: x2 = x*s_x; s_x2 = max|x2|/127; qx = round(clip(x2/s_x2)))
    x2 = (x * s_x).astype(np.float32)
    s_x2 = (np.abs(x2).max() / np.float32(127.0)).astype(np.float32)
    qx = np.round(np.clip((x2 / s_x2).astype(np.float32), -128.0, 127.0))
    qxt = np.ascontiguousarray(qx.reshape(N_CORES, M, D).transpose(0, 2, 1))
    qxt = qxt.astype(np.float16)  # [cores, D, M], int8-valued: exact in fp16
    s1 = (s_w1 * s_x2).astype(np.float32)

    b1s1 = np.ascontiguousarray(
        (np.asarray(b1, np.float32) * s1).reshape(NH, 128).T
    ).astype(np.float32)  # [128, NH]
    b2q = np.asarray(b2, np.float32).reshape(1, D).astype(np.float16)
    scal = np.array([[0.0, s1, s_w2, 0.0]], np.float32)

    in_maps = []
    for c in range(N_CORES):
        in_maps.append(
            {
                "qxt": qxt[c],
                "w1t": w1t,
                "w2t": w2t,
                "b1s1": b1s1,
                "b2q": b2q,
                "scal": scal,
            }
        )
    return in_maps


_NC_CACHE = {}


def get_nc(collectives=True, **_ignored):
    key = collectives
    if key not in _NC_CACHE:
        _NC_CACHE[key] = build_nc(collectives=collectives)
    return _NC_CACHE[key]


def kernel(x, act_scaling_factor, w1, b1, w2, b2):
    in_maps = prep_inputs(x, act_scaling_factor, w1, b1, w2, b2)
    nc = get_nc()
    res = bass_utils.run_bass_kernel_spmd(
        nc, in_maps, core_ids=list(range(N_CORES)), trace=False
    )
    outs = [res.results[c]["out"] for c in range(N_CORES)]
    full = np.concatenate(outs, axis=0).reshape(B, S, D).astype(np.float32)
    return full


if __name__ == "__main__":
    rng = np.random.RandomState(0)
    inputs = {
        "x": rng.randn(B, S, D).astype(np.float32),
        "act_scaling_factor": np.ones(1, np.float32),
        "w1": (rng.randn(H, D) / np.sqrt(D)).astype(np.float32),
        "b1": (0.02 * rng.randn(H)).astype(np.float32),
        "w2": (rng.randn(D, H) / np.sqrt(H)).astype(np.float32),
        "b2": (0.02 * rng.randn(D)).astype(np.float32),
    }
    out = kernel(**inputs)
    print("out", out.shape, out.dtype, float(np.abs(out).max()))


# revision 28
# speedup vs baseline: 1.2241x; 1.1267x over previous
"""Trainium2 Bass kernel for nn_Mlp_45449343926805 (quantized MLP, 8 cores).

Strategy (v3):
- Data-parallel over batch: x [128,197,384] -> 8 shards of 3152 tokens.
- Everything that is a pure elementwise/layout function of the inputs with a
  host-known scale is done on host (numpy), exactly as the reference does:
  weight quantization, the x activation quant scale, AND the x quantization +
  transpose (qxT shipped as fp16 [D, M] per shard). int8-valued operands are
  exact in fp16.
- Device graph per core:
    GEMM1 (h^T = qw1 @ qxT) with fused bias+GELU out of 4-bank PSUM tiles,
    h stored fp16; |h|max via fused tensor_tensor_reduce; one device
    AllReduce(max) for the h quant scale (the only cross-core dependency);
    requantize h with the fp16 magic-rounding trick (1536.0: fp32->fp16 RNE
    conversion lands exactly on the integer grid, round-half-to-even ==
    jnp.round); GEMM2 with qh^T slices stationary, fc2 bias as a K=1
    ones-row matmul; out = psum * s2 via ScalarE; DMA out.
- A dummy warmup AllReduce at kernel start absorbs collective bring-up and
  launch skew while weights/qxT stream in.
"""

import sys

if "/opt/trn_rl_repo" not in sys.path:
    sys.path.insert(0, "/opt/trn_rl_repo")

import numpy as np

import concourse.bass as bass  # noqa: F401  (registers arch bits)
import concourse.mybir as mybir
import concourse.tile as tile
from concourse import bacc
from concourse import bass_utils

N_CORES = 8
B, S, D, H = 128, 197, 384, 1536
M = (B // N_CORES) * S  # tokens per core = 3152
KD = D // 128  # 3 contraction tiles for fc1
NH = H // 128  # 12 h tiles (also fc2 contraction tiles)
MAGIC = 1536.0  # 1.5*2^10: fp16 round-to-integer magic
R127 = 1.0 / 127.0

FP32 = mybir.dt.float32
FP16 = mybir.dt.float16

# token tiles (partition dim, <=128)
TOKS = [(t, min(128, M - t)) for t in range(0, M, 128)]  # 25
# 512-token chunks (PSUM-bank-width output slices)
CHUNKS = [(c, min(512, M - c)) for c in range(0, M, 512)]  # 7 (last=80)
# GEMM1 blocks: up to 4 chunks -> one [128,2048] psum tile
BLOCKS = [CHUNKS[i : i + 4] for i in range(0, len(CHUNKS), 4)]
# GEMM2 psum groups: up to 4 token tiles -> one [128,2048] psum tile
PSGROUPS = [TOKS[i : i + 4] for i in range(0, len(TOKS), 4)]

AX = mybir.AxisListType.X
OP = mybir.AluOpType
AF = mybir.ActivationFunctionType
GELU_FN = AF.Gelu  # sim_test swaps for Tanh (CoreSim lacks Gelu)
WARMUP_CC = True
SAFE_GROUPS = True  # k-inner contiguous accumulation groups per bank
SAFE_ACT = False  # GELU reads PSUM per 512-col bank slice
USE_TTR = False  # tensor_tensor_reduce for hmax vs plain tensor_reduce


def build_nc(collectives: bool = True):
    nc = bacc.Bacc(
        "TRN2", target_bir_lowering=False, debug=False, num_devices=N_CORES
    )
    qxt_in = nc.dram_tensor("qxt", [D, M], FP16, kind="ExternalInput")
    w1t_in = nc.dram_tensor("w1t", [D, H], FP16, kind="ExternalInput")
    w2t_in = nc.dram_tensor("w2t", [H, D], FP16, kind="ExternalInput")
    b1s1_in = nc.dram_tensor("b1s1", [128, NH], FP32, kind="ExternalInput")
    b2q_in = nc.dram_tensor("b2q", [1, D], FP16, kind="ExternalInput")
    sc_in = nc.dram_tensor("scal", [1, 4], FP32, kind="ExternalInput")
    out = nc.dram_tensor("out", [M, D], FP32, kind="ExternalOutput")

    with tile.TileContext(nc) as tc:
        with (
            tc.tile_pool(name="persist", bufs=1) as pp,
            tc.tile_pool(name="stage", bufs=3) as st,
            tc.tile_pool(name="small", bufs=1) as sm,
            tc.tile_pool(name="ps", bufs=2, space="PSUM") as ps,
            tc.tile_pool(name="dram", bufs=2, space="DRAM") as dram,
        ):
            # ---- persistent weights / constants ----
            # scalar queue: needed earliest (w1t, b1s1, scal)
            w1t_sb = []
            for k in range(KD):
                w1t_k = pp.tile([128, H], FP16, name=f"w1t_{k}", tag=f"w1t_{k}")
                nc.scalar.dma_start(w1t_k[:], w1t_in[128 * k : 128 * (k + 1), :])
                w1t_sb.append(w1t_k)
            b1s1_sb = pp.tile([128, NH], FP32)
            nc.scalar.dma_start(b1s1_sb[:], b1s1_in[:, :])
            sc_row = pp.tile([1, 4], FP32)
            nc.scalar.dma_start(sc_row[:], sc_in[:, :])
            sc_bc = pp.tile([128, 4], FP32)
            nc.gpsimd.partition_broadcast(sc_bc[:], sc_row[:])
            s1_c = sc_bc[:, 1:2]
            sw2_c = sc_bc[:, 2:3]

            # ---- warmup collective: absorb CC bring-up + launch skew ----
            # issued before the bulk DMAs so it runs concurrently with them
            if collectives and WARMUP_CC:
                wrm = sm.tile([1, 8], FP32, tag="wrm")
                nc.vector.memset(wrm[:], 0.0)
                warm_in = dram.tile([8], FP32, tag="warm_in")
                warm_out = dram.tile([8], FP32, tag="warm_out")
                nc.sync.dma_start(warm_in[:], wrm[0, :])
                nc.gpsimd.collective_compute(
                    "AllReduce",
                    OP.max,
                    replica_groups=[list(range(N_CORES))],
                    ins=[warm_in[:].opt()],
                    outs=[warm_out[:].opt()],
                )

            # sync queue: qxT in 1024-col pieces (k-interleaved so GEMM1's
            # first block unblocks early), then fc2-side weights
            qxT = [
                pp.tile([128, M], FP16, name=f"qxT_{k}", tag=f"qxT_{k}")
                for k in range(KD)
            ]
            for c0 in range(0, M, 1024):
                cw = min(1024, M - c0)
                for k in range(KD):
                    nc.sync.dma_start(
                        qxT[k][:, c0 : c0 + cw],
                        qxt_in[128 * k : 128 * (k + 1), c0 : c0 + cw],
                    )
            w2t_sb = []
            for k in range(NH):
                w2t_k = pp.tile([128, D], FP16, name=f"w2t_{k}", tag=f"w2t_{k}")
                nc.sync.dma_start(w2t_k[:], w2t_in[128 * k : 128 * (k + 1), :])
                w2t_sb.append(w2t_k)
            b2q_sb = pp.tile([1, D], FP16)
            nc.sync.dma_start(b2q_sb[:], b2q_in[:, :])
            ones1 = pp.tile([1, 128], FP16)
            nc.vector.memset(ones1[:], 1.0)

            h_sb = [
                pp.tile([128, M], FP16, name=f"h_{j}", tag=f"h_{j}")
                for j in range(NH)
            ]
            hmax_cols = pp.tile([128, 24], FP32)

            # ---- GEMM1: h^T = qw1 @ qxT, fused bias+GELU, h max ----
            col = 0
            for blk in BLOCKS:
                bc0 = blk[0][0]
                bw = sum(cw for _, cw in blk)
                for j in range(NH):
                    psum = ps.tile(
                        [128, 2048], FP32, name=f"ps1_{bc0}_{j}", tag="mm", bufs=2
                    )
                    if SAFE_GROUPS:
                        for ci, (c0, cw) in enumerate(blk):
                            for k in range(KD):
                                nc.tensor.matmul(
                                    psum[:, 512 * ci : 512 * ci + cw],
                                    w1t_sb[k][:, 128 * j : 128 * (j + 1)],
                                    qxT[k][:, c0 : c0 + cw],
                                    start=(k == 0),
                                    stop=(k == KD - 1),
                                )
                    else:
                        for k in range(KD):
                            for ci, (c0, cw) in enumerate(blk):
                                nc.tensor.matmul(
                                    psum[:, 512 * ci : 512 * ci + cw],
                                    w1t_sb[k][:, 128 * j : 128 * (j + 1)],
                                    qxT[k][:, c0 : c0 + cw],
                                    start=(k == 0),
                                    stop=(k == KD - 1),
                                )
                    if SAFE_ACT:
                        for ci, (c0, cw) in enumerate(blk):
                            nc.scalar.activation(
                                h_sb[j][:, c0 : c0 + cw],
                                psum[:, 512 * ci : 512 * ci + cw], GELU_FN,
                                bias=b1s1_sb[:, j : j + 1], scale=s1_c,
                            )
                    else:
                        nc.scalar.activation(
                            h_sb[j][:, bc0 : bc0 + bw], psum[:, :bw], GELU_FN,
                            bias=b1s1_sb[:, j : j + 1], scale=s1_c,
                        )
                    half = bw // 2
                    if USE_TTR:
                        hscr = st.tile(
                            [128, 1024], FP16, name=f"hs_{bc0}_{j}", tag="hs",
                            bufs=2,
                        )
                        # GELU(x) >= -0.17 and the positive max is >> 0.17 for
                        # this model, so max(h) == max(|h|) exactly.
                        nc.vector.tensor_tensor_reduce(
                            hscr[:, :half],
                            h_sb[j][:, bc0 : bc0 + half],
                            h_sb[j][:, bc0 + half : bc0 + bw],
                            1.0,
                            0.0,
                            op0=OP.max,
                            op1=OP.max,
                            accum_out=hmax_cols[:, col : col + 1],
                        )
                    else:
                        nc.vector.tensor_reduce(
                            hmax_cols[:, col : col + 1],
                            h_sb[j][:, bc0 : bc0 + bw],
                            axis=AX, op=OP.max, apply_absolute_value=True,
                        )
                    col += 1

            # ---- AllReduce(max) for the h quant scale ----
            hred = sm.tile([128, 1], FP32, tag="hred")
            nc.vector.tensor_reduce(hred[:], hmax_cols[:, :col], axis=AX, op=OP.max)
            gh = _allreduce_max(nc, dram, sm, hred, collectives, "h")
            s_h = sm.tile([128, 1], FP32, tag="s_h")
            nc.vector.tensor_scalar(s_h[:], gh[:], R127, None, op0=OP.mult)
            i2 = sm.tile([128, 1], FP32, tag="i2")
            nc.vector.reciprocal(i2[:], s_h[:])

            s2 = sm.tile([128, 1], FP32, tag="s2")
            nc.vector.tensor_scalar(s2[:], s_h[:], sw2_c, None, op0=OP.mult)

            # ---- GEMM2: requantize h (fp16 magic), out = (qh@qw2 + b2)*s2 ----
            qh_cur = {}
            cur_chunk = -1
            for pg in PSGROUPS:
                psum = ps.tile(
                    [128, 2048], FP32, name=f"ps2_{pg[0][0]}", tag="mm", bufs=2
                )
                for slot, (t0, tw) in enumerate(pg):
                    g = t0 // 512
                    if g != cur_chunk:
                        c0, cw = CHUNKS[g]
                        qh_cur = {}
                        for k in range(NH):
                            p2 = st.tile(
                                [128, 512], FP16, name=f"p2_{g}_{k}", tag="p2",
                                bufs=4,
                            )
                            nc.vector.tensor_scalar(
                                p2[:, :cw], h_sb[k][:, c0 : c0 + cw],
                                i2[:, 0:1], MAGIC, op0=OP.mult, op1=OP.add,
                            )
                            qh = st.tile(
                                [128, 512], FP16, name=f"qh_{g}_{k}", tag="qh",
                                bufs=24,
                            )
                            nc.vector.tensor_scalar(
                                qh[:, :cw], p2[:, :cw], MAGIC, None,
                                op0=OP.subtract,
                            )
                            qh_cur[k] = qh
                        cur_chunk = g
                    loc = t0 - CHUNKS[g][0]
                    for k in range(NH):
                        nc.tensor.matmul(
                            psum[:tw, 512 * slot : 512 * slot + D],
                            qh_cur[k][:, loc : loc + tw],
                            w2t_sb[k][:, :],
                            start=(k == 0),
                            stop=False,
                        )
                    nc.tensor.matmul(
                        psum[:tw, 512 * slot : 512 * slot + D],
                        ones1[:, :tw],
                        b2q_sb[:, :],
                        start=False,
                        stop=True,
                    )
                    osb = st.tile(
                        [128, D], FP32, name=f"o_{t0}", tag="o", bufs=4
                    )
                    nc.scalar.activation(
                        osb[:tw, :], psum[:tw, 512 * slot : 512 * slot + D],
                        AF.Copy, bias=0.0, scale=s2[:tw, 0:1],
                    )
                    nc.sync.dma_start(out[t0 : t0 + tw, :], osb[:tw, :])

    nc.compile()
    return nc


def _allreduce_max(nc, dram, sm, vec_col, collectives, name):
    """AllReduce(max) of a [128,1] fp32 column across cores.
    Returns a [128,1] tile holding the global max in every partition."""
    cc_in = dram.tile([128], FP32, name=f"ccin_{name}", tag=f"ccin_{name}")
    nc.sync.dma_start(cc_in[:], vec_col[:, 0])
    if collectives:
        # AllGather + local max: skips the reduce-scatter phase of AllReduce
        cc_out = dram.tile(
            [128 * N_CORES], FP32, name=f"ccout_{name}", tag=f"ccout_{name}"
        )
        nc.gpsimd.collective_compute(
            "AllGather",
            OP.bypass,
            replica_groups=[list(range(N_CORES))],
            ins=[cc_in[:].opt()],
            outs=[cc_out[:].opt()],
        )
        row = sm.tile(
            [1, 128 * N_CORES], FP32, name=f"ccrow_{name}", tag=f"ccrow_{name}"
        )
        nc.sync.dma_start(row[:], cc_out[:])
    else:
        cc_out = dram.tile([128], FP32, name=f"ccout_{name}", tag=f"ccout_{name}")
        nc.sync.dma_start(cc_out[:], cc_in[:])
        row = sm.tile([1, 128], FP32, name=f"ccrow_{name}", tag=f"ccrow_{name}")
        nc.sync.dma_start(row[:], cc_out[:])
    g1 = sm.tile([1, 1], FP32, name=f"ccg1_{name}", tag=f"ccg1_{name}")
    nc.vector.tensor_reduce(g1[:], row[:], axis=AX, op=OP.max)
    gbc = sm.tile([128, 1], FP32, name=f"ccgb_{name}", tag=f"ccgb_{name}")
    nc.gpsimd.partition_broadcast(gbc[:], g1[:])
    return gbc


# ---------------- host side ----------------

def _quant_weight(w):
    w = np.asarray(w, np.float32)
    s = (np.abs(w).max() / np.float32(127.0)).astype(np.float32)
    q = np.clip(np.round((w / s).astype(np.float32)), -128.0, 127.0)
    return q.astype(np.float32), s


def prep_inputs(x, act_scaling_factor, w1, b1, w2, b2):
    x = np.asarray(x, np.float32).reshape(-1, D)
    s_x = np.float32(np.asarray(act_scaling_factor).reshape(-1)[0])
    qw1, s_w1 = _quant_weight(w1)
    qw2, s_w2 = _quant_weight(w2)
    w1t = np.ascontiguousarray(qw1.T).astype(np.float16)  # [D, H]
    w2t = np.ascontiguousarray(qw2.T).astype(np.float16)  # [H, D]

    # host-side x quantization (pure function of inputs, exactly as the
    # reference: x2 = x*s_x; s_x2 = max|x2|/127; qx = round(clip(x2/s_x2)))
    x2 = (x * s_x).astype(np.float32)
    s_x2 = (np.abs(x2).max() / np.float32(127.0)).astype(np.float32)
    qx = np.round(np.clip((x2 / s_x2).astype(np.float32), -128.0, 127.0))
    qxt = np.ascontiguousarray(qx.reshape(N_CORES, M, D).transpose(0, 2, 1))
    qxt = qxt.astype(np.float16)  # [cores, D, M], int8-valued: exact in fp16
    s1 = (s_w1 * s_x2).astype(np.float32)

    b1s1 = np.ascontiguousarray(
        (np.asarray(b1, np.float32) * s1).reshape(NH, 128).T
    ).astype(np.float32)  # [128, NH]
    b2q = np.asarray(b2, np.float32).reshape(1, D).astype(np.float16)
    scal = np.array([[0.0, s1, s_w2, 0.0]], np.float32)

    in_maps = []
    for c in range(N_CORES):
        in_maps.append(
            {
                "qxt": qxt[c],
                "w1t": w1t,
                "w2t": w2t,
                "b1s1": b1s1,
                "b2q": b2q,
                "scal": scal,
            }
        )
    return in_maps


_NC_CACHE = {}


def get_nc(collectives=True, **_ignored):
    key = collectives
    if key not in _NC_CACHE:
        _NC_CACHE[key] = build_nc(collectives=collectives)
    return _NC_CACHE[key]


def kernel(x, act_scaling_factor, w1, b1, w2, b2):
    in_maps = prep_inputs(x, act_scaling_factor, w1, b1, w2, b2)
    nc = get_nc()
    res = bass_utils.run_bass_kernel_spmd(
        nc, in_maps, core_ids=list(range(N_CORES)), trace=False
    )
    outs = [res.results[c]["out"] for c in range(N_CORES)]
    full = np.concatenate(outs, axis=0).reshape(B, S, D).astype(np.float32)
    return full


if __name__ == "__main__":
    rng = np.random.RandomState(0)
    inputs = {
        "x": rng.randn(B, S, D).astype(np.float32),
        "act_scaling_factor": np.ones(1, np.float32),
        "w1": (rng.randn(H, D) / np.sqrt(D)).astype(np.float32),
        "b1": (0.02 * rng.randn(H)).astype(np.float32),
        "w2": (rng.randn(D, H) / np.sqrt(H)).astype(np.float32),
        "b2": (0.02 * rng.randn(D)).astype(np.float32),
    }
    out = kernel(**inputs)
    print("out", out.shape, out.dtype, float(np.abs(out).max()))
